# revision 17
# baseline (speedup 1.0000x reference)
"""Trainium2 Bass kernel for nn_Mix8Net (dense directional-conv CNN).

Data-parallel over 8 NeuronCores: batch 1024 -> 128 samples/core.

Per-core dataflow (channels on SBUF partitions, pixels*samples on free dim):
  - Activations live in a "tall" layout: per-sample block = 16 rows x 16
    cols = 256 fp32 (row 0 = zero separator, col 0 = zero border, 15x15
    image at rows 1-15 / cols 1-15).  A 3-tap directional conv tap with
    offset (dy,dx) is a matmul whose moving operand is the per-sample
    flat range [block, block+512) (2 samples) shifted by 16*dy+dx -
    out-of-image reads land on the zero separators/borders.
  - Matmuls run in bf16 (1 cycle/row, FWL-able weight loads) accumulating
    fp32 in PSUM.  Each matmul writes a full 512-f32 PSUM bank (2 x 256).
  - The scalar engine (ACT) is the bottleneck of this network (11 silu
    layers x 1 elem/cycle/lane), so silu is applied to the 225 interior
    pixels only, 8 samples (4 banks, uniform 256 stride) per instruction:
    [p, s8, 15, 15].  Borders of the trunk buffers are therefore never
    written and stay zero from the one-time init - no fixup passes.
  - Residual adds run on VectorE over interior views; the final conv's
    bias-add (Identity) also runs on VectorE straight out of PSUM, so ACT
    does nothing but the 11 silus.
  - The initial conv (CIN=2, 3 taps) is packed into one K=6 matmul per
    sample pair using host-prepared pre-shifted copies of x, placed at
    partitions 32*d (one group per direction).
"""
import numpy as np
import concourse.bacc as bacc
import concourse.mybir as mybir
import concourse.tile as tile
from concourse import bass_utils

F32 = mybir.dt.float32
BF16 = mybir.dt.float16
NPBF16 = np.float16
AF = mybir.ActivationFunctionType

DIR_OFFSETS = (
    ((0, -1), (0, 0), (0, 1)),
    ((-1, 0), (0, 0), (1, 0)),
    ((-1, -1), (0, 0), (1, 1)),
    ((1, -1), (0, 0), (-1, 1)),
)

B, CIN, H, W = 1024, 2, 15, 15
M, COUT = 128, 64
NB = 4                      # DirectionalConvResBlocks
NCORES = 8
BLOC = B // NCORES          # 128 samples per core
NS = 16                     # samples per chunk
NCH = BLOC // NS            # 8 chunks
BLK = 256                   # 16x16 per-sample block
TALLB = (NS + 2) * BLK      # chunk window incl lead/trail blocks
TOT = (BLOC + 2) * BLK      # full-core tall array length

_CACHE = {}


def _mov(buf, s0, delta=0, psl=None):
    """[p, 2, 240] moving view: rows 1-15 of 2 sample blocks from s0."""
    a = BLK * (s0 + 1) + 16 + delta
    v = buf[:, a:a + 512] if psl is None else buf[psl, a:a + 512]
    return v.rearrange("p (s q) -> p s q", q=BLK)[:, :, 0:240]


def _int8(buf, s0):
    """[p, 8, 15, 15] interior view of 8 sample blocks from sample s0."""
    a = BLK * (s0 + 1)
    v = buf[:, a:a + 8 * BLK]
    return v.rearrange("p (s r c) -> p s r c", r=16, c=16)[:, :, 1:16, 1:16]


def _ps_int(ps, np_=128):
    """[p, 8, 15, 15] interior view of a 4-bank psum tile (8 samples)."""
    v = ps[0:np_].rearrange("p b q -> p (b q)")
    return v.rearrange("p (s r c) -> p s r c", r=16, c=16)[:, :, 1:16, 1:16]


def _build(nch=NCH, af=None, trunk=None):
    af = AF.Silu if af is None else af
    TRK = BF16 if trunk is None else trunk
    nc = bacc.Bacc("TRN2", target_bir_lowering=False, debug=False)

    xt12_d = nc.dram_tensor("xt12", [4, 6, TOT], TRK, kind="ExternalInput").ap()
    wd0_d = nc.dram_tensor("wd0", [6, 128], BF16, kind="ExternalInput").ap()
    wdc_d = nc.dram_tensor("wdc", [128, 12, 128], BF16, kind="ExternalInput").ap()
    wpx_d = nc.dram_tensor("wpx", [128, NB, 128], BF16, kind="ExternalInput").ap()
    wc1_d = nc.dram_tensor("wc1", [128, 128], BF16, kind="ExternalInput").ap()
    wc2_d = nc.dram_tensor("wc2", [128, 128], BF16, kind="ExternalInput").ap()
    wf_d = nc.dram_tensor("wf", [128, 64], BF16, kind="ExternalInput").ap()
    bd0_d = nc.dram_tensor("bd0", [128, 1], F32, kind="ExternalInput").ap()
    bdc_d = nc.dram_tensor("bdc", [128, NB], F32, kind="ExternalInput").ap()
    bpx_d = nc.dram_tensor("bpx", [128, NB], F32, kind="ExternalInput").ap()
    bc1_d = nc.dram_tensor("bc1", [128, 1], F32, kind="ExternalInput").ap()
    bc2_d = nc.dram_tensor("bc2", [128, 1], F32, kind="ExternalInput").ap()
    bf_d = nc.dram_tensor("bf", [64, 1], F32, kind="ExternalInput").ap()
    out_d = nc.dram_tensor("out", [BLOC, 4, COUT, 225], F32,
                           kind="ExternalOutput")

    with tile.TileContext(nc) as tc:
        # persistent SBUF tensors
        xsb = [nc.alloc_sbuf_tensor(f"xs{i}", [128, TALLB], TRK).ap()
               for i in range(2)]
        t1b = [nc.alloc_sbuf_tensor(f"t1{i}", [128, TALLB], BF16).ap()
               for i in range(2)]
        t2b = [nc.alloc_sbuf_tensor(f"t2{i}", [128, TALLB], BF16).ap()
               for i in range(2)]
        xt = nc.alloc_sbuf_tensor("xt", [128, TALLB], TRK).ap()
        ofb = [nc.alloc_sbuf_tensor(f"of{i}", [64, NS * 225], F32).ap()
               for i in range(2)]
        wd0 = nc.alloc_sbuf_tensor("wd0s", [128, 128], BF16).ap()
        wdc = nc.alloc_sbuf_tensor("wdcs", [128, 12, 128], BF16).ap()
        wpx = nc.alloc_sbuf_tensor("wpxs", [128, NB, 128], BF16).ap()
        wc1 = nc.alloc_sbuf_tensor("wc1s", [128, 128], BF16).ap()
        wc2 = nc.alloc_sbuf_tensor("wc2s", [128, 128], BF16).ap()
        wf = nc.alloc_sbuf_tensor("wfs", [128, 64], BF16).ap()
        bd0 = nc.alloc_sbuf_tensor("bd0s", [128, 1], F32).ap()
        bdc = nc.alloc_sbuf_tensor("bdcs", [128, NB], F32).ap()
        bpx = nc.alloc_sbuf_tensor("bpxs", [128, NB], F32).ap()
        bc1 = nc.alloc_sbuf_tensor("bc1s", [128, 1], F32).ap()
        bc2 = nc.alloc_sbuf_tensor("bc2s", [128, 1], F32).ap()
        bf = nc.alloc_sbuf_tensor("bfs", [64, 1], F32).ap()

        with tc.tile_pool(name="psum", bufs=2, space="PSUM") as pspool:
            # one-time zeroing (t1b too: px conv reads its full
            # 512-ranges incl. the never-written borders)
            for xs in xsb + t1b:
                nc.vector.memset(xs[:], 0.0)
            for d in range(4):
                nc.sync.dma_start(wd0[32 * d:32 * d + 6, 0:128], wd0_d)
            nc.sync.dma_start(wdc[:], wdc_d)
            nc.sync.dma_start(wpx[:], wpx_d)
            nc.sync.dma_start(wc1[:], wc1_d)
            nc.sync.dma_start(wc2[:], wc2_d)
            nc.sync.dma_start(wf[:], wf_d)
            for t_, d_ in [(bd0, bd0_d), (bdc, bdc_d), (bpx, bpx_d),
                           (bc1, bc1_d), (bc2, bc2_d), (bf, bf_d)]:
                nc.sync.dma_start(t_[:], d_)

            def conv_phase(jobs):
                """One conv layer for a pair of directions, interleaved at
                tile granularity so each direction's PE fill overlaps the
                other's ACT drain.  job = (src, wap, bias_ap, dst, deltas,
                psl, tile_position, res)."""
                for t in range(2):
                    for (src, wap, bias_ap, dst, deltas, psl, tp,
                         res) in jobs:
                        ps = pspool.tile([128, 4, 512], F32, tag="ps")
                        for g in range(4):
                            s0 = 8 * t + 2 * g
                            pso = ps[:, g, 0:512].rearrange(
                                "p (s q) -> p s q", q=BLK)[:, :, 16:256]
                            for ti, dl in enumerate(deltas):
                                nc.tensor.matmul(
                                    pso,
                                    wap if len(deltas) == 1
                                    else wap[:, ti, :],
                                    _mov(src, s0, dl, psl=psl),
                                    start=(ti == 0),
                                    stop=(ti == len(deltas) - 1),
                                    tile_position=tp,
                                )
                        nc.scalar.activation(_int8(dst, 8 * t), _ps_int(ps),
                                             af, bias=bias_ap)
                        if res is not None:
                            nc.vector.tensor_add(_int8(res, 8 * t),
                                                 _int8(res, 8 * t),
                                                 _int8(dst, 8 * t))

            for ch in range(nch):
                a0 = BLK * NS * ch
                for d in range(4):
                    nc.sync.dma_start(xt[32 * d:32 * d + 6, :],
                                      xt12_d[d, :, a0:a0 + TALLB])
                # tile-level interleave of the two directions of a pair:
                # each direction's heavy PE fill overlaps the other's ACT
                # drain
                for pair in ((0, 1), (2, 3)):
                    sl = {d: i for i, d in enumerate(pair)}
                    conv_phase([
                        (xt, wd0[slice(32 * d, 32 * d + 6), 0:128], bd0[:],
                         xsb[sl[d]], [0], slice(32 * d, 32 * d + 6),
                         (32 * d, 0), None)
                        for d in pair])
                    for i in range(NB):
                        conv_phase([
                            (xsb[sl[d]], wdc[:, 3 * i:3 * i + 3, :],
                             bdc[:, i:i + 1], t1b[sl[d]],
                             [16 * dy + dx for (dy, dx) in DIR_OFFSETS[d]],
                             None, None, None)
                            for d in pair])
                        conv_phase([
                            (t1b[sl[d]], wpx[:, i, :], bpx[:, i:i + 1],
                             t2b[sl[d]], [0], None, None, xsb[sl[d]])
                            for d in pair])
                    conv_phase([
                        (xsb[sl[d]], wc1[:], bc1[:], t1b[sl[d]], [0],
                         None, None, None) for d in pair])
                    conv_phase([
                        (t1b[sl[d]], wc2[:], bc2[:], t2b[sl[d]], [0],
                         None, None, xsb[sl[d]]) for d in pair])
                    # final conv (COUT=64): bias-add on DVE out of PSUM
                    for t in range(2):
                        for d in pair:
                            xs = xsb[sl[d]]
                            of = ofb[sl[d]]
                            ps = pspool.tile([128, 4, 512], F32, tag="ps")
                            for g in range(4):
                                s0 = 8 * t + 2 * g
                                pso = ps[0:64, g, 0:512].rearrange(
                                    "p (s q) -> p s q", q=BLK)[:, :, 16:256]
                                nc.tensor.matmul(
                                    pso, wf[:],
                                    _mov(xs, s0), start=True, stop=True)
                            ofv = of[:, 1800 * t:1800 * (t + 1)].rearrange(
                                "p (s r c) -> p s r c", r=15, c=15)
                            nc.vector.tensor_scalar_add(
                                ofv, _ps_int(ps, 64), bf[:])
                    for d in pair:
                        srcv = ofb[sl[d]][:].rearrange("p (s q) -> p s q",
                                                       q=225)
                        dst = out_d.ap()[NS * ch:NS * ch + NS, d]\
                            .transpose((1, 0, 2))
                        nc.sync.dma_start(dst, srcv)

    nc.compile()
    return nc


def _prep(x, w_d0, b_d0, w_dc, b_dc, w_px, b_px, w_c1, b_c1, w_c2, b_c2,
          w_f, b_f, np_trunk=NPBF16):
    """Host-side packing: weights transposed to lhsT, x pre-shifted per
    direction/tap into the tall layout."""
    x = np.asarray(x, np.float32)

    # tall per-core x: [core, 2, TOT]
    xtall = np.zeros((NCORES, CIN, BLOC + 2, 16, 16), np.float32)
    xs = x.reshape(NCORES, BLOC, CIN, H, W)
    xtall[:, :, 1:BLOC + 1, 1:16, 1:16] = xs.transpose(0, 2, 1, 3, 4)
    xtall = xtall.reshape(NCORES, CIN, TOT)

    xt12 = np.zeros((NCORES, 4, 6, TOT), np.float32)
    for d in range(4):
        for t in range(3):
            dy, dx = DIR_OFFSETS[d][t]
            dl = 16 * dy + dx
            for c in range(CIN):
                srcv = xtall[:, c]
                dst = xt12[:, d, 2 * t + c]
                if dl > 0:
                    dst[:, :-dl] = srcv[:, dl:]
                elif dl < 0:
                    dst[:, -dl:] = srcv[:, :dl]
                else:
                    dst[:] = srcv

    bfc = lambda a: np.ascontiguousarray(a).astype(NPBF16)
    com = dict(
        wd0=bfc(np.asarray(w_d0, np.float32).transpose(0, 2, 1).reshape(6, 128)),
        wdc=bfc(np.asarray(w_dc, np.float32).transpose(3, 0, 1, 2).reshape(128, 12, 128)),
        wpx=bfc(np.asarray(w_px, np.float32).transpose(2, 0, 1)),
        wc1=bfc(np.asarray(w_c1, np.float32).T),
        wc2=bfc(np.asarray(w_c2, np.float32).T),
        wf=bfc(np.asarray(w_f, np.float32).T),
        bd0=np.asarray(b_d0, np.float32).reshape(128, 1),
        bdc=np.ascontiguousarray(np.asarray(b_dc, np.float32).T),
        bpx=np.ascontiguousarray(np.asarray(b_px, np.float32).T),
        bc1=np.asarray(b_c1, np.float32).reshape(128, 1),
        bc2=np.asarray(b_c2, np.float32).reshape(128, 1),
        bf=np.asarray(b_f, np.float32).reshape(64, 1),
    )
    in_maps = []
    for core in range(NCORES):
        m = dict(com)
        m["xt12"] = xt12[core].astype(np_trunk)
        in_maps.append(m)
    return in_maps


LAST_RESULT = None


def kernel(**inputs) -> np.ndarray:
    global LAST_RESULT
    if "nc" not in _CACHE:
        _CACHE["nc"] = _build()
    nc = _CACHE["nc"]
    in_maps = _prep(**inputs)
    res = bass_utils.run_bass_kernel_spmd(nc, in_maps,
                                          core_ids=list(range(NCORES)))
    LAST_RESULT = res
    out = np.concatenate([r["out"] for r in res.results], axis=0)
    return np.ascontiguousarray(out.reshape(B, 4, COUT, H, W))
